# revision 23
# baseline (speedup 1.0000x reference)
"""2-layer GAT on 8 trn2 NeuronCores (Bass/Tile).

Strategy (matches the sharding hint): nodes are partitioned across the 8
cores (12500 each, padded to 12544 = 98*128), dealt round-robin in global
degree order so every core's tile k has a near-identical degree profile
(minimizes the shared-schedule padding).  Edges are assigned to the core
owning their destination.  Three SPMD launches:

  1. "build":  h1 = x @ W1 plus the attention projections, written as a
     per-node table T1 ([64 h | 8 s] fp16) -- each core builds its own
     node slice from its slice of x^T.
  2. "layer1": the host performs the halo exchange (all-to-all gather of
     remote source-node rows into per-edge streams, pure replication /
     indexing -- no math); the device streams them in sequentially at full
     DMA bandwidth, forms z = s[src] + a[dst] (a streamed per edge),
     el = exp(max(z, 0.2 z)), weights the messages, and segment-sums via
     banded 0/1 S-matrices on the PE (PSUM band accumulation, one PSUM
     batch per <=7 dst tiles).  A batched epilogue normalizes the softmax,
     applies ELU, and projects through the fused [W2 | W2 a_src2 |
     W2 a_dst2] to emit the 17-column layer-2 table T2 = [16 p2 | 1 as2]
     plus per-node ad2.
  3. "layer2": same machinery on the (tiny) T2 streams, aggregation
     directly in the 16-dim projected space, then log_softmax.

Between launches the host only concatenates / replicates device-computed
arrays (the halo exchange): T1/T2 slices -> full tables -> per-edge
streams, per-node a values -> per-edge streams.  All model math runs on
device.
"""

import numpy as np
import ml_dtypes

import concourse.bacc as bacc
import concourse.tile as tile
import concourse.mybir as mybir
from concourse import bass_utils

F32 = mybir.dt.float32
F16 = mybir.dt.float16
F8 = mybir.dt.float8e4
AF = mybir.ActivationFunctionType
ALU = mybir.AluOpType
AX = mybir.AxisListType

# problem constants (hardcoded per the task statement)
NCORES = 8
N = 100000
IN = 256
HID = 8
HEADS = 8
OUT = 16
NEG = 0.2
NPC = 12500            # real nodes per core
MPC = 12544            # padded nodes per core (98 * 128)
NT = MPC // 128        # 98 dst tiles per core
BATCH_EDGES = 16384    # shared edge budget per batch
NBMAX = 7              # max dst tiles per batch (PSUM bank limit)
NROWS = NCORES * MPC   # 100352 table rows
EPS = 1e-16
SLAB1 = 14             # launch-1 chunks per slab (must divide NT)

NCOLS1 = 72            # T1 row: 64 h | 8 s
NCOLS2 = 18            # T2 row: 16 p2 | as2 | ad2 (pad)
MW1 = 72               # launch2 matmul rhs: 64 msg | 8 el
MW2 = 18               # launch3 matmul rhs: 16 msg | el | el dup

import os
FINLVL = int(os.environ.get("FINLVL", "9"))
_CACHE = {}
TRACE = False            # set True to capture HW profiles (exec_time_ns)
DBG = "full"             # debug: "edge" | "matmul" | "full"


# --------------------------------------------------------------------------
# host-side graph preprocessing (pure index work)
# --------------------------------------------------------------------------

def _preprocess(edge_index):
    src = np.concatenate([np.asarray(edge_index[0]), np.arange(N)])
    dst = np.concatenate([np.asarray(edge_index[1]), np.arange(N)])
    deg = np.bincount(dst, minlength=N)

    # global degree-sorted round-robin dealing: node order[i] -> core i%8,
    # rank i//8.  Every core's rank-r node then has ~the same degree, so the
    # shared (max-over-cores) schedule has minimal padding.
    order = np.argsort(-deg, kind="stable")
    i = np.arange(N)
    pos = np.empty(N, np.int64)
    pos[order] = (i % NCORES) * MPC + (i // NCORES)
    perm_nodes = np.full(NROWS, -1, np.int64)   # table row -> node id (or -1)
    perm_nodes[(i % NCORES) * MPC + (i // NCORES)] = order

    srcpos = pos[src]
    dstpos = pos[dst]

    cores = []
    # per-(core, tile) counts; shared per-tile offsets across cores keep the
    # chunk->tile structure identical on every core (tight union schedule).
    # Tiles are packed into batches by edge budget and a hard NBMAX tile cap
    # (PSUM epilogue batching).
    counts = np.zeros((NCORES, NT), np.int64)
    per_core = []
    for c in range(NCORES):
        m = (dstpos >= c * MPC) & (dstpos < c * MPC + NPC)
        sp = srcpos[m]
        rank = dstpos[m] - c * MPC
        t = rank // 128
        per_core.append((sp, rank, t))
        np.add.at(counts[c], t, 1)
    stc = counts.max(0)                              # [NT]
    bmap = np.zeros(NT, np.int64)
    acc = 0
    ntl = 0
    b = 0
    for t in range(NT):
        if acc and (acc + stc[t] > BATCH_EDGES or ntl >= NBMAX):
            b += 1
            acc = 0
            ntl = 0
        bmap[t] = b
        acc += stc[t]
        ntl += 1
    NBAT = int(bmap[-1]) + 1
    btiles = [list(np.where(bmap == bb)[0]) for bb in range(NBAT)]
    # shared tile offsets within each batch
    toff = np.zeros(NT, np.int64)
    gsz = np.zeros(NBAT, np.int64)
    for bb in range(NBAT):
        off = 0
        for t in btiles[bb]:
            toff[t] = off
            off += stc[t]
        gsz[bb] = off
    G = np.maximum((gsz + 127) // 128, 1)            # [NBAT] slabs
    Q = G * 128
    qoff = np.concatenate([[0], np.cumsum(Q)])       # [NBAT+1]
    goff = qoff // 128
    TOTQ = int(qoff[-1])
    TOTG = TOTQ // 128

    # per-core padded position arrays
    for c in range(NCORES):
        sp, rank, t = per_core[c]
        b = bmap[t]
        order2 = np.lexsort((rank, t))
        sp, rank, t, b = sp[order2], rank[order2], t[order2], b[order2]
        # within-tile index
        gstart = np.searchsorted(t, np.arange(NT), side="left")
        within = np.arange(len(t)) - gstart[t]
        q = qoff[b] + toff[t] + within
        cores.append({"sp": sp, "rank": rank, "b": b, "q": q})

    # union matmul schedule, merged per (b, t, j) with a band range.
    JMAX = TOTG + 1
    keysets = []
    for c in range(NCORES):
        d = cores[c]
        j = (d["q"] - qoff[d["b"]]) // 128
        t = d["rank"] // 128
        a = (d["rank"] % 128) // 32
        key = t * JMAX + j
        keysets.append((key, a))
        d["j"] = j
        d["t"] = t
        d["key"] = key
    allk = np.concatenate([k for k, _ in keysets])
    alla = np.concatenate([a for _, a in keysets])
    ukeys, inv = np.unique(allk, return_inverse=True)
    TOTB = len(ukeys)
    amin = np.full(TOTB, 4, np.int64)
    amax = np.full(TOTB, -1, np.int64)
    np.minimum.at(amin, inv, alla)
    np.maximum.at(amax, inv, alla)
    # band -> (col base, width) in 32-partition units, PE-tile-aligned
    ecol = np.where(amin == amax, amin,
                    np.where((amin == 0) & (amax == 1), 0,
                             np.where((amin == 2) & (amax == 3), 2, 0)))
    ewid = np.where(amin == amax, 1,
                    np.where((amin == 0) & (amax == 1), 2,
                             np.where((amin == 2) & (amax == 3), 2, 4)))
    soff = np.concatenate([[0], np.cumsum(ewid)])   # block col offsets (32u)
    # decode (b, t, j)
    uj = ukeys % JMAX
    ut = ukeys // JMAX
    ub = bmap[ut]
    sched = {"b": ub, "t": ut, "j": uj, "col": ecol, "wid": ewid,
             "soff": soff, "n": TOTB, "totw": int(soff[-1])}

    # per-core S fill data (entry id + in-chunk row + in-block col per edge)
    for c in range(NCORES):
        d = cores[c]
        ent = np.searchsorted(ukeys, d["key"])
        d["ent"] = ent
        d["k"] = d["q"] % 128
        d["scol"] = d["rank"] % 128 - ecol[ent] * 32

    meta = {"G": G, "qoff": qoff, "goff": goff, "TOTQ": TOTQ,
            "TOTG": TOTG, "sched": sched, "pos": pos, "NBAT": NBAT,
            "btiles": btiles, "perm_nodes": perm_nodes, "cores": cores}
    return meta


def _build_s(meta):
    """Per-core fp8 S blocks, stream replication maps."""
    TOTQ = meta["TOTQ"]
    s_all, streams, spq_all = [], [], []
    for c in range(NCORES):
        d = meta["cores"][c]
        soff = meta["sched"]["soff"]
        totw = meta["sched"]["totw"]
        S = np.zeros((128, totw * 32), ml_dtypes.float8_e4m3)
        S[d["k"], soff[d["ent"]] * 32 + d["scol"]] = 1.0
        s_all.append(S)

        # per-position (p, g, rank) for the a streams
        streams.append((d["q"] % 128, d["q"] // 128, d["rank"]))

        # per-position source table row (0 for padding)
        spq = np.zeros(TOTQ, np.int64)
        spq[d["q"]] = d["sp"]
        spq_all.append(spq)
    return s_all, streams, spq_all


def _expand_stream(stream, r_core, width, totg):
    """r_core [MPC, width] f32 -> per-position [128, totg, width] f16."""
    p, g, rank = stream
    out = np.zeros((128, int(totg), width), np.float16)
    out[p, g, :] = r_core[rank, :width].astype(np.float16)
    return out


def _expand_rows(spq, table, width):
    """table [NROWS, width] f16 -> per-position stream [128, TOTG*width]."""
    rows = table[spq]                                # [TOTQ, width]
    return np.ascontiguousarray(
        rows.reshape(-1, 128, width).transpose(1, 0, 2)).reshape(128, -1)


# --------------------------------------------------------------------------
# launch builders
# --------------------------------------------------------------------------

def _new_nc():
    return bacc.Bacc("TRN2", target_bir_lowering=False, debug=False,
                     enable_asserts=False, num_devices=NCORES)


def _build_launch1():
    nc = _new_nc()
    xs_d = nc.dram_tensor("xs", [IN, MPC], F16, kind="ExternalInput")
    wc_d = nc.dram_tensor("wc", [IN, 80], F16, kind="ExternalInput")
    t1_d = nc.dram_tensor("t1s", [128, NT * NCOLS1], F16,
                          kind="ExternalOutput")
    r1_d = nc.dram_tensor("r1", [128, NT * 8], F32, kind="ExternalOutput")
    SLAB = SLAB1
    with tile.TileContext(nc) as tc:
        with tc.tile_pool(name="w", bufs=1) as wp, \
             tc.tile_pool(name="x", bufs=3) as xp, \
             tc.tile_pool(name="o", bufs=3) as op, \
             tc.tile_pool(name="ps", bufs=4, space="PSUM") as pp:
            wc_sb = wp.tile([128, 2, 80], F16)
            nc.sync.dma_start(wc_sb[:, 0, :], wc_d.ap()[0:128, :])
            nc.sync.dma_start(wc_sb[:, 1, :], wc_d.ap()[128:256, :])
            for s in range(NT // SLAB):
                cols = slice(s * SLAB * 128, (s + 1) * SLAB * 128)
                xt0 = xp.tile([128, SLAB * 128], F16, tag="xt0")
                xt1 = xp.tile([128, SLAB * 128], F16, tag="xt1")
                nc.sync.dma_start(xt0[:], xs_d.ap()[0:128, cols])
                nc.sync.dma_start(xt1[:], xs_d.ap()[128:256, cols])
                tout = op.tile([128, SLAB, NCOLS1], F16, tag="tout")
                rout = op.tile([128, SLAB, 8], F32, tag="rout")
                for half, (c0, c1) in enumerate(((0, 6), (6, 12), (12, SLAB))):
                    psb = pp.tile([128, c1 - c0, 80], F32, tag=f"ps{half}",
                                  bufs=2)
                    for i in range(c0, c1):
                        nc.tensor.matmul(
                            psb[:, i - c0, :],
                            lhsT=xt0[:, i * 128:(i + 1) * 128],
                            rhs=wc_sb[:, 0, :], start=True, stop=False,
                            skip_group_check=True)
                        nc.tensor.matmul(
                            psb[:, i - c0, :],
                            lhsT=xt1[:, i * 128:(i + 1) * 128],
                            rhs=wc_sb[:, 1, :], start=False, stop=True,
                            skip_group_check=True)
                    nc.vector.tensor_copy(tout[:, c0:c1, :],
                                          psb[:, :, 0:NCOLS1])
                    nc.scalar.copy(rout[:, c0:c1, :], psb[:, :, 72:80])
                c0 = s * SLAB
                nc.sync.dma_start(
                    t1_d.ap()[:, c0 * NCOLS1:(c0 + SLAB) * NCOLS1],
                    tout[:].rearrange("p i f -> p (i f)"))
                nc.scalar.dma_start(
                    r1_d.ap()[:, c0 * 8:(c0 + SLAB) * 8],
                    rout[:].rearrange("p i f -> p (i f)"))
    nc.compile()
    return nc


def _emit_msg_layer(nc, tc, meta, ms_d, s_d, ae_d, finalize,
                    rwidth, tabw, mwidth):
    """Shared structure of launches 2/3.

    rwidth: per-edge a-stream width (8 for L1, 1 for L2); tabw: stream row
    width; mwidth: matmul rhs width (= hcols + rwidth).  `finalize`
    supplies the per-edge elementwise ops and the per-batch epilogue.
    """
    G, qoff, goff = meta["G"], meta["qoff"], meta["goff"]
    sched = meta["sched"]
    sb, st, sj = (sched[k] for k in ("b", "t", "j"))
    scol, swid, soff = sched["col"], sched["wid"], sched["soff"]
    TOTB = sched["n"]
    ent_by_t = {}
    for i in range(TOTB):
        ent_by_t.setdefault(int(st[i]), []).append(i)
    NBAT = meta["NBAT"]
    btiles = meta["btiles"]
    blo = np.searchsorted(sb, np.arange(NBAT))
    bhi = np.searchsorted(sb, np.arange(NBAT), side="right")
    # batch S-column ranges (32-unit blocks)
    slo = [int(soff[blo[b]]) for b in range(NBAT)]
    shi = [int(soff[bhi[b]]) for b in range(NBAT)]
    nw32max = max(1, max(shi[b] - slo[b] for b in range(NBAT)))
    gmax = int(G.max())

    with tc.tile_pool(name="resident", bufs=1) as rp, \
         tc.tile_pool(name="gslab", bufs=3) as gp, \
         tc.tile_pool(name="mslab", bufs=1) as mp, \
         tc.tile_pool(name="work", bufs=3) as wkp, \
         tc.tile_pool(name="fin", bufs=3) as fp, \
         tc.tile_pool(name="psA", bufs=3, space="PSUM") as ppA, \
         tc.tile_pool(name="psB", bufs=2, space="PSUM") as ppB:
        pools = (rp, gp, mp, wkp, fp, ppA, ppB)
        zrow = rp.tile([1, 128], F16)
        nc.vector.memset(zrow[:], 0.0)
        zwide = rp.tile([1, 512], F16)
        nc.vector.memset(zwide[:], 0.0)
        cst_sb = finalize.load_consts(nc, rp)
        # finalize may aggregate SUPER consecutive batches into one PSUM
        # group (one epilogue pass); all tiles of a group are consecutive
        SUPER = getattr(finalize, "SUPER", 1)
        grp_of = [b // SUPER for b in range(NBAT)]
        NGRP = grp_of[-1] + 1
        gtiles = [sum((btiles[b] for b in range(NBAT) if grp_of[b] == gg),
                      []) for gg in range(NGRP)]
        snbmax = max(len(ts) for ts in gtiles)
        pending = None
        ps = None
        for b in range(NBAT):
            nb = len(btiles[b])
            gg = grp_of[b]
            g = int(G[b])
            g0 = int(goff[b])
            nw32 = max(shi[b] - slo[b], 1)
            ssb = mp.tile([128, nw32max, 32], F8, tag="s", bufs=3)
            if shi[b] > slo[b]:
                nc.sync.dma_start(
                    ssb[:, 0:nw32, :],
                    s_d.ap()[:, slo[b] * 32:shi[b] * 32]
                    .rearrange("p (n c) -> p n c", c=32))
            if b == 0 or grp_of[b - 1] != gg:
                snb = len(gtiles[gg])
                gt0 = int(gtiles[gg][0])
                ps = ppA.tile([128, snbmax, mwidth], F32, tag="ps")
                nc.tensor.matmul(
                    ps[:, 0:snb, :].rearrange("p n m -> p (n m)"),
                    lhsT=zrow[:], rhs=zwide[:, 0:snb * mwidth],
                    start=True, stop=False, skip_group_check=True)
            # per-edge message stream (host halo exchange), full bandwidth
            Gs = gp.tile([128, gmax, mwidth], F16, tag="G", bufs=3)
            nc.sync.dma_start(
                Gs[:, 0:g, 0:tabw],
                ms_d.ap()[:, g0 * tabw:(g0 + g) * tabw]
                .rearrange("p (g r) -> p g r", r=tabw))
            rs = gp.tile([128, gmax, rwidth], F16, tag="rs")
            nc.sync.dma_start(
                rs[:, 0:g, :], ae_d.ap()[:, g0 * rwidth:(g0 + g) * rwidth]
                .rearrange("p (g r) -> p g r", r=rwidth))
            if DBG != "stream":
                finalize.edge_ops(nc, Gs, rs, wkp, g)
            if DBG in ("stream", "edge"):
                continue
            # previous group's epilogue goes between this batch's edge ops
            # and its matmuls: the DVE queue never stalls on the PE
            if DBG == "full" and pending is not None:
                finalize.batch_ops(nc, pools, *pending, cst_sb)
                pending = None
            # matmuls, tile-major within the batch
            for t in btiles[b]:
                i = int(t) - gt0
                ents = ent_by_t.get(t, [])
                for n, e in enumerate(ents):
                    j = int(sj[e])
                    col, wid = int(scol[e]), int(swid[e])
                    so = int(soff[e]) - slo[b]
                    nc.tensor.matmul(
                        ps[col * 32:(col + wid) * 32, i, :],
                        lhsT=ssb[:, so:so + wid, :]
                        .rearrange("p n c -> p (n c)"),
                        rhs=Gs[:, j, :],
                        start=False, stop=(n == len(ents) - 1),
                        tile_position=(0, col * 32),
                        skip_group_check=True)
            if b == NBAT - 1 or grp_of[b + 1] != gg:
                pending = (gg, gt0, snb, ps)
        if DBG == "full" and pending is not None:
            finalize.batch_ops(nc, pools, *pending, cst_sb)
        if DBG == "full" and hasattr(finalize, "finish"):
            finalize.finish(nc)


class _L1Final:
    """Layer-1 epilogue: softmax normalize, ELU, project through
    [W2 | W2 a_src2 | W2 a_dst2] to build the 17-col T2 row + ad2."""

    def __init__(self, nc, w2e_d, id_d, t2_d, r2_d):
        self.w2e_d, self.id_d = w2e_d, id_d
        self.t2_d, self.r2_d = t2_d, r2_d
        self.r2_sb = None

    def load_consts(self, nc, rp):
        # w2e replicated in both partition halves (matmul lhsT/rhs must
        # share a base partition)
        w2e = rp.tile([128, 18], F16)
        nc.sync.dma_start(w2e[0:64, :], self.w2e_d.ap())
        nc.sync.dma_start(w2e[64:128, :], self.w2e_d.ap())
        idm = rp.tile([128, 128], F32)
        nc.sync.dma_start(idm[:], self.id_d.ap())
        self.r2_sb = rp.tile([128, NT], F32)
        nc.vector.memset(self.r2_sb[:], 0.0)
        return (w2e, idm)

    def finish(self, nc):
        nc.scalar.dma_start(
            self.r2_d.ap().rearrange("(t p) o -> p (t o)", p=128), self.r2_sb[:])

    def edge_ops(self, nc, Gs, rs, wkp, g):
        # z = s[src] + a[dst]; el = exp(max(z, 0.2 z)); in-place h *= el.
        # z/zl run on the (otherwise idle) GPSIMD; the big multiply keeps
        # all last dims packed (h block is channel-major) for 2x DVE mode.
        z = wkp.tile([128, Gs.shape[1], 8], F16, tag="z")
        nc.vector.tensor_tensor(out=z[:, 0:g], in0=Gs[:, 0:g, 64:72],
                                in1=rs[:, 0:g], op=ALU.add)
        zl = wkp.tile([128, Gs.shape[1], 8], F16, tag="zl")
        nc.vector.tensor_scalar_mul(zl[:, 0:g], z[:, 0:g], NEG)
        nc.vector.tensor_tensor(out=zl[:, 0:g], in0=z[:, 0:g],
                                in1=zl[:, 0:g], op=ALU.max)
        nc.scalar.activation(out=Gs[:, 0:g, 64:72], in_=zl[:, 0:g],
                             func=AF.Exp)
        gh = (g + 1) // 2
        for lo, hi in ((0, gh), (gh, g)):
            gl = hi - lo
            nc.vector.tensor_tensor(
                out=Gs[:, lo:hi, 0:64].rearrange("p g (c h) -> p g c h", c=8),
                in0=Gs[:, lo:hi, 0:64].rearrange("p g (c h) -> p g c h", c=8),
                in1=Gs[:, lo:hi, 64:72].unsqueeze(2)
                .broadcast_to([128, gl, 8, 8]),
                op=ALU.mult)

    def batch_ops(self, nc, pools, gg, t0, nb, ps, consts):
        rp, gp, mp, wkp, fp, ppA, ppB = pools
        w2e, idm = consts
        den = fp.tile([128, NBMAX, 8], F32, tag="den")
        nc.vector.tensor_scalar_add(den[:, 0:nb], ps[:, 0:nb, 64:72], EPS)
        rec = fp.tile([128, NBMAX, 8], F32, tag="rec")
        nc.vector.reciprocal(rec[:, 0:nb], den[:, 0:nb])
        y = fp.tile([128, NBMAX, 64], F32, tag="y")
        nc.vector.tensor_tensor(
            out=y[:, 0:nb].rearrange("p n (c h) -> p n c h", c=8),
            in0=ps[:, 0:nb, 0:64].rearrange("p n (c h) -> p n c h", c=8),
            in1=rec[:, 0:nb].unsqueeze(2).broadcast_to([128, nb, 8, 8]),
            op=ALU.mult)
        # ELU: t2h = max(y, exp(min(y,0)) - 1)
        yn = fp.tile([128, NBMAX, 64], F32, tag="yn")
        nc.vector.tensor_scalar_min(yn[:, 0:nb], y[:, 0:nb], 0.0)
        ey = fp.tile([128, NBMAX, 64], F32, tag="ey")
        nc.scalar.activation(out=ey[:, 0:nb], in_=yn[:, 0:nb], func=AF.Exp)
        nc.vector.tensor_scalar_add(ey[:, 0:nb], ey[:, 0:nb], -1.0)
        t2h = fp.tile([128, NBMAX, 64], F32, tag="t2h")
        nc.vector.tensor_tensor(out=t2h[:, 0:nb], in0=y[:, 0:nb],
                                in1=ey[:, 0:nb], op=ALU.max)
        if FINLVL < 2:
            return
        # project tiles: transpose then @ [W2 | w2a | w2d]
        ps2 = ppB.tile([128, NBMAX, 18], F32, tag="ps2", bufs=2)
        for i in range(nb):
            tp = ppB.tile([64, 128], F32, tag="tp", bufs=2)
            nc.tensor.transpose(tp[:], t2h[:, i, :], idm[:])
            agT = fp.tile([64, 128], F16, tag="agT")
            nc.scalar.copy(agT[:], tp[:])
            if FINLVL < 3:
                continue
            nc.tensor.matmul(ps2[:, i, :], lhsT=agT[:], rhs=w2e[0:64, :],
                             start=True, stop=True, skip_group_check=True)
        if FINLVL < 4:
            return
        t2t = fp.tile([128, NBMAX, NCOLS2], F16, tag="t2t")
        nc.vector.tensor_copy(t2t[:, 0:nb, :], ps2[:, 0:nb, 0:18])
        nc.scalar.copy(self.r2_sb[:, t0:t0 + nb], ps2[:, 0:nb, 17])
        rows = slice(t0 * 128, (t0 + nb) * 128)
        nc.sync.dma_start(
            self.t2_d.ap()[rows, :].rearrange("(i p) f -> p i f", p=128),
            t2t[:, 0:nb, :])


class _L2Final:
    """Layer-2 epilogue: normalize (already in W2-projected space),
    log_softmax.  SUPER consecutive batches share one PSUM group (28 x 18
    f32 rows fill exactly one PSUM bank), so the epilogue chain runs 4x
    less often and the output writes in >=512B fragments."""

    SUPER = 4
    SNB = NBMAX * SUPER

    def __init__(self, nc, o_d):
        self.o_d = o_d

    def load_consts(self, nc, rp):
        return None

    def edge_ops(self, nc, Gs, rs, wkp, g):
        # el duplicated into cols 16 and 17 so the 16-wide multiply keeps
        # packed last dims (2x DVE mode)
        z = wkp.tile([128, Gs.shape[1], 1], F16, tag="z")
        nc.vector.tensor_tensor(out=z[:, 0:g], in0=Gs[:, 0:g, 16:17],
                                in1=rs[:, 0:g], op=ALU.add)
        zl = wkp.tile([128, Gs.shape[1], 1], F16, tag="zl")
        nc.vector.tensor_scalar_mul(zl[:, 0:g], z[:, 0:g], NEG)
        nc.vector.tensor_tensor(out=zl[:, 0:g], in0=z[:, 0:g],
                                in1=zl[:, 0:g], op=ALU.max)
        nc.scalar.activation(out=Gs[:, 0:g, 16:17], in_=zl[:, 0:g],
                             func=AF.Exp)
        nc.scalar.activation(out=Gs[:, 0:g, 17:18], in_=zl[:, 0:g],
                             func=AF.Exp)
        nc.vector.tensor_tensor(
            out=Gs[:, 0:g, 0:16].rearrange("p g (u v) -> p g u v", v=2),
            in0=Gs[:, 0:g, 0:16].rearrange("p g (u v) -> p g u v", v=2),
            in1=Gs[:, 0:g, 16:18].unsqueeze(2).broadcast_to([128, g, 8, 2]),
            op=ALU.mult)

    def batch_ops(self, nc, pools, gg, t0, nb, ps, consts):
        rp, gp, mp, wkp, fp, ppA, ppB = pools
        SNB = self.SNB
        # den > 0 (self-loops); logits are small, so log_softmax needs no
        # max subtraction
        rec = fp.tile([128, SNB, 1], F32, tag="rec2")
        nc.vector.reciprocal(rec[:, 0:nb], ps[:, 0:nb, 16:17])
        o1 = fp.tile([128, SNB, 16], F32, tag="o1")
        nc.vector.tensor_tensor(
            out=o1[:, 0:nb], in0=ps[:, 0:nb, 0:16],
            in1=rec[:, 0:nb].rearrange("p n o -> p (n o)")
            .to_broadcast([128, nb, 16]),
            op=ALU.mult)
        es = fp.tile([128, SNB, 16], F16, tag="es")
        nc.scalar.activation(out=es[:, 0:nb], in_=o1[:, 0:nb], func=AF.Exp)
        ssum = fp.tile([128, SNB, 1], F32, tag="ssum")
        nc.vector.tensor_reduce(out=ssum[:, 0:nb], in_=es[:, 0:nb], axis=AX.X,
                                op=ALU.add)
        lns = fp.tile([128, SNB, 1], F32, tag="lns")
        nc.scalar.activation(out=lns[:, 0:nb], in_=ssum[:, 0:nb], func=AF.Ln)
        res = fp.tile([128, SNB, 16], F32, tag="res")
        nc.vector.tensor_tensor(
            out=res[:, 0:nb], in0=o1[:, 0:nb],
            in1=lns[:, 0:nb].rearrange("p n o -> p (n o)")
            .to_broadcast([128, nb, 16]),
            op=ALU.subtract)
        nc.sync.dma_start(
            self.o_d.ap()[:, t0 * 16:(t0 + nb) * 16],
            res[:, 0:nb, :].rearrange("p n o -> p (n o)"))


def _build_launch2(meta):
    nc = _new_nc()
    ms_d = nc.dram_tensor("ms1", [128, meta["TOTG"] * NCOLS1], F16,
                          kind="ExternalInput")
    s_d = nc.dram_tensor("sall", [128, meta["sched"]["totw"] * 32], F8,
                         kind="ExternalInput")
    ae_d = nc.dram_tensor("ae1", [128, meta["TOTG"] * 8], F16,
                          kind="ExternalInput")
    w2e_d = nc.dram_tensor("w2e", [64, 18], F16, kind="ExternalInput")
    id_d = nc.dram_tensor("idm", [128, 128], F32, kind="ExternalInput")
    t2_d = nc.dram_tensor("t2s", [MPC, NCOLS2], F16, kind="ExternalOutput")
    r2_d = nc.dram_tensor("r2", [MPC, 1], F32, kind="ExternalOutput")
    fin = _L1Final(nc, w2e_d, id_d, t2_d, r2_d)
    with tile.TileContext(nc) as tc:
        _emit_msg_layer(nc, tc, meta, ms_d, s_d, ae_d, fin,
                        rwidth=8, tabw=NCOLS1, mwidth=MW1)
    nc.compile()
    return nc


def _build_launch3(meta):
    nc = _new_nc()
    ms_d = nc.dram_tensor("ms2", [128, meta["TOTG"] * NCOLS2], F16,
                          kind="ExternalInput")
    s_d = nc.dram_tensor("sall", [128, meta["sched"]["totw"] * 32], F8,
                         kind="ExternalInput")
    ae_d = nc.dram_tensor("ae2", [128, meta["TOTG"] * 1], F16,
                          kind="ExternalInput")
    o_d = nc.dram_tensor("o", [128, NT * 16], F32, kind="ExternalOutput")
    fin = _L2Final(nc, o_d)
    with tile.TileContext(nc) as tc:
        _emit_msg_layer(nc, tc, meta, ms_d, s_d, ae_d, fin,
                        rwidth=1, tabw=NCOLS2, mwidth=MW2)
    nc.compile()
    return nc


# --------------------------------------------------------------------------
# the kernel
# --------------------------------------------------------------------------

def kernel(x, edge_index, W1, a_src1, a_dst1, b1, W2, a_src2, a_dst2, b2):
    x = np.asarray(x, np.float32)
    edge_index = np.asarray(edge_index)
    W1 = np.asarray(W1, np.float32)
    W2 = np.asarray(W2, np.float32)
    a_src1 = np.asarray(a_src1, np.float32)
    a_dst1 = np.asarray(a_dst1, np.float32)
    a_src2 = np.asarray(a_src2, np.float32)
    a_dst2 = np.asarray(a_dst2, np.float32)

    key = edge_index.tobytes()[:4096]
    if _CACHE.get("key") != key:
        meta = _preprocess(edge_index)
        s_all, streams, spq_all = _build_s(meta)
        _CACHE.update(key=key, meta=meta, s_all=s_all, streams=streams,
                      spq_all=spq_all,
                      nc1=_build_launch1(), nc2=_build_launch2(meta),
                      nc3=_build_launch3(meta))
    meta = _CACHE["meta"]
    s_all, streams, spq_all = (_CACHE["s_all"], _CACHE["streams"],
                               _CACHE["spq_all"])

    # weight packing.  The h blocks live channel-major ((c, h) index) on
    # device so the per-edge el broadcast keeps packed last dims; reorder
    # W1 columns / W2 rows correspondingly here.
    W1r = W1.reshape(IN, HEADS, HID)
    B1 = np.einsum("khc,hc->kh", W1r, a_src1)        # [256, 8]
    C1 = np.einsum("khc,hc->kh", W1r, a_dst1)
    W1cm = W1r.transpose(0, 2, 1).reshape(IN, 64)
    wc = np.concatenate([W1cm, B1, C1], 1).astype(np.float16)  # [256, 80]
    W2cm = W2.reshape(HEADS, HID, OUT).transpose(1, 0, 2).reshape(64, OUT)
    w2a = W2cm @ a_src2[0]                            # [64]
    w2d = W2cm @ a_dst2[0]
    w2e = np.concatenate([W2cm, w2a[:, None], w2d[:, None]],
                         1).astype(np.float16)        # [64, 18]
    idm = np.eye(128, dtype=np.float32)

    # launch 1: build T1 slices
    perm = meta["perm_nodes"]
    xT = np.zeros((IN, NROWS), np.float16)
    real = perm >= 0
    xT[:, real] = x[perm[real]].astype(np.float16).T
    in1 = [{"xs": np.ascontiguousarray(xT[:, c * MPC:(c + 1) * MPC]),
            "wc": wc} for c in range(NCORES)]
    r1_res = bass_utils.run_bass_kernel_spmd(
        _CACHE["nc1"], in1, core_ids=list(range(NCORES)), trace=TRACE)
    T1 = np.concatenate(
        [r1_res.results[c]["t1s"].reshape(128, NT, NCOLS1)
         .transpose(1, 0, 2).reshape(MPC, NCOLS1) for c in range(NCORES)], 0)

    # launch 2: layer-1 message passing (host halo exchange -> streams)
    in2 = []
    for c in range(NCORES):
        ms1 = _expand_rows(spq_all[c], T1, NCOLS1)
        r1c = (r1_res.results[c]["r1"].reshape(128, NT, 8)
               .transpose(1, 0, 2).reshape(MPC, 8))
        ae1 = _expand_stream(streams[c], r1c, 8, meta["TOTG"])
        in2.append({"ms1": ms1, "sall": s_all[c],
                    "ae1": ae1.reshape(128, -1), "w2e": w2e, "idm": idm})
    r2_res = bass_utils.run_bass_kernel_spmd(
        _CACHE["nc2"], in2, core_ids=list(range(NCORES)), trace=TRACE)
    T2 = np.concatenate([r2_res.results[c]["t2s"] for c in range(NCORES)], 0)

    # launch 3: layer-2 + head
    in3 = []
    for c in range(NCORES):
        ms2 = _expand_rows(spq_all[c], T2, NCOLS2)
        ae2 = _expand_stream(streams[c], r2_res.results[c]["r2"], 1,
                             meta["TOTG"])
        in3.append({"ms2": ms2, "sall": s_all[c],
                    "ae2": ae2.reshape(128, -1)})
    r3_res = bass_utils.run_bass_kernel_spmd(
        _CACHE["nc3"], in3, core_ids=list(range(NCORES)), trace=TRACE)
    o_all = np.concatenate(
        [r3_res.results[c]["o"].reshape(128, NT, 16)
         .transpose(1, 0, 2).reshape(MPC, 16) for c in range(NCORES)], 0)

    out = o_all[meta["pos"][np.arange(N)]].astype(np.float32)
    _CACHE["exec_ns"] = [r.exec_time_ns for r in (r1_res, r2_res, r3_res)]
    _CACHE["profiles"] = [r.profile_json for r in (r1_res, r2_res, r3_res)]
    _CACHE["traces"] = [r.instructions_and_trace
                        for r in (r1_res, r2_res, r3_res)]
    return out


def predict_ns():
    """Cost-model (TimelineSim) per-launch predictions for cached programs."""
    from concourse.timeline_sim import TimelineSim
    out = []
    for k in ("nc1", "nc2", "nc3"):
        out.append(TimelineSim(_CACHE[k]).simulate())
    return out


# revision 24
# speedup vs baseline: 1.0419x; 1.0419x over previous
"""2-layer GAT on 8 trn2 NeuronCores (Bass/Tile).

Strategy (matches the sharding hint): nodes are partitioned across the 8
cores (12500 each, padded to 12544 = 98*128), dealt round-robin in global
degree order so every core's tile k has a near-identical degree profile
(minimizes the shared-schedule padding).  Edges are assigned to the core
owning their destination.  Three SPMD launches:

  1. "build":  h1 = x @ W1 plus the attention projections, written as a
     per-node table T1 ([64 h | 8 s] fp16) -- each core builds its own
     node slice from its slice of x^T.
  2. "layer1": the host performs the halo exchange (all-to-all gather of
     remote source-node rows into per-edge streams, pure replication /
     indexing -- no math); the device streams them in sequentially at full
     DMA bandwidth, forms z = s[src] + a[dst] (a streamed per edge),
     el = exp(max(z, 0.2 z)), weights the messages, and segment-sums via
     banded 0/1 S-matrices on the PE (PSUM band accumulation, one PSUM
     batch per <=7 dst tiles).  A batched epilogue normalizes the softmax,
     applies ELU, and projects through the fused [W2 | W2 a_src2 |
     W2 a_dst2] to emit the 17-column layer-2 table T2 = [16 p2 | 1 as2]
     plus per-node ad2.
  3. "layer2": same machinery on the (tiny) T2 streams, aggregation
     directly in the 16-dim projected space, then log_softmax.

Between launches the host only concatenates / replicates device-computed
arrays (the halo exchange): T1/T2 slices -> full tables -> per-edge
streams, per-node a values -> per-edge streams.  All model math runs on
device.
"""

import numpy as np
import ml_dtypes

import concourse.bacc as bacc
import concourse.tile as tile
import concourse.mybir as mybir
from concourse import bass_utils

F32 = mybir.dt.float32
F16 = mybir.dt.float16
F8 = mybir.dt.float8e4
AF = mybir.ActivationFunctionType
ALU = mybir.AluOpType
AX = mybir.AxisListType

# problem constants (hardcoded per the task statement)
NCORES = 8
N = 100000
IN = 256
HID = 8
HEADS = 8
OUT = 16
NEG = 0.2
NPC = 12500            # real nodes per core
MPC = 12544            # padded nodes per core (98 * 128)
NT = MPC // 128        # 98 dst tiles per core
BATCH_EDGES = 16384    # shared edge budget per batch
NBMAX = 7              # max dst tiles per batch (PSUM bank limit)
NROWS = NCORES * MPC   # 100352 table rows
EPS = 1e-16
SLAB1 = 14             # launch-1 chunks per slab (must divide NT)

NCOLS1 = 72            # T1 row: 64 h | 8 s
NCOLS2 = 18            # T2 row: 16 p2 | as2 | ad2 (pad)
MW1 = 72               # launch2 matmul rhs: 64 msg | 8 el
MW2 = 18               # launch3 matmul rhs: 16 msg | el | el dup

import os
FINLVL = int(os.environ.get("FINLVL", "9"))
_CACHE = {}
TRACE = False            # set True to capture HW profiles (exec_time_ns)
DBG = "full"             # debug: "edge" | "matmul" | "full"


# --------------------------------------------------------------------------
# host-side graph preprocessing (pure index work)
# --------------------------------------------------------------------------

def _preprocess(edge_index):
    src = np.concatenate([np.asarray(edge_index[0]), np.arange(N)])
    dst = np.concatenate([np.asarray(edge_index[1]), np.arange(N)])
    deg = np.bincount(dst, minlength=N)

    # global degree-sorted round-robin dealing: node order[i] -> core i%8,
    # rank i//8.  Every core's rank-r node then has ~the same degree, so the
    # shared (max-over-cores) schedule has minimal padding.
    order = np.argsort(-deg, kind="stable")
    i = np.arange(N)
    pos = np.empty(N, np.int64)
    pos[order] = (i % NCORES) * MPC + (i // NCORES)
    perm_nodes = np.full(NROWS, -1, np.int64)   # table row -> node id (or -1)
    perm_nodes[(i % NCORES) * MPC + (i // NCORES)] = order

    srcpos = pos[src]
    dstpos = pos[dst]

    cores = []
    # per-(core, tile) counts; shared per-tile offsets across cores keep the
    # chunk->tile structure identical on every core (tight union schedule).
    # Tiles are packed into batches by edge budget and a hard NBMAX tile cap
    # (PSUM epilogue batching).
    counts = np.zeros((NCORES, NT), np.int64)
    per_core = []
    for c in range(NCORES):
        m = (dstpos >= c * MPC) & (dstpos < c * MPC + NPC)
        sp = srcpos[m]
        rank = dstpos[m] - c * MPC
        t = rank // 128
        per_core.append((sp, rank, t))
        np.add.at(counts[c], t, 1)
    stc = counts.max(0)                              # [NT]
    bmap = np.zeros(NT, np.int64)
    acc = 0
    ntl = 0
    b = 0
    for t in range(NT):
        if acc and (acc + stc[t] > BATCH_EDGES or ntl >= NBMAX):
            b += 1
            acc = 0
            ntl = 0
        bmap[t] = b
        acc += stc[t]
        ntl += 1
    NBAT = int(bmap[-1]) + 1
    btiles = [list(np.where(bmap == bb)[0]) for bb in range(NBAT)]
    # shared tile offsets within each batch
    toff = np.zeros(NT, np.int64)
    gsz = np.zeros(NBAT, np.int64)
    for bb in range(NBAT):
        off = 0
        for t in btiles[bb]:
            toff[t] = off
            off += stc[t]
        gsz[bb] = off
    G = np.maximum((gsz + 127) // 128, 1)            # [NBAT] slabs
    Q = G * 128
    qoff = np.concatenate([[0], np.cumsum(Q)])       # [NBAT+1]
    goff = qoff // 128
    TOTQ = int(qoff[-1])
    TOTG = TOTQ // 128

    # per-core padded position arrays
    for c in range(NCORES):
        sp, rank, t = per_core[c]
        b = bmap[t]
        order2 = np.lexsort((rank, t))
        sp, rank, t, b = sp[order2], rank[order2], t[order2], b[order2]
        # within-tile index
        gstart = np.searchsorted(t, np.arange(NT), side="left")
        within = np.arange(len(t)) - gstart[t]
        q = qoff[b] + toff[t] + within
        cores.append({"sp": sp, "rank": rank, "b": b, "q": q})

    # union matmul schedule, merged per (b, t, j) with a band range.
    JMAX = TOTG + 1
    keysets = []
    for c in range(NCORES):
        d = cores[c]
        j = (d["q"] - qoff[d["b"]]) // 128
        t = d["rank"] // 128
        a = (d["rank"] % 128) // 32
        key = t * JMAX + j
        keysets.append((key, a))
        d["j"] = j
        d["t"] = t
        d["key"] = key
    allk = np.concatenate([k for k, _ in keysets])
    alla = np.concatenate([a for _, a in keysets])
    ukeys, inv = np.unique(allk, return_inverse=True)
    TOTB = len(ukeys)
    amin = np.full(TOTB, 4, np.int64)
    amax = np.full(TOTB, -1, np.int64)
    np.minimum.at(amin, inv, alla)
    np.maximum.at(amax, inv, alla)
    # band -> (col base, width) in 32-partition units, PE-tile-aligned
    ecol = np.where(amin == amax, amin,
                    np.where((amin == 0) & (amax == 1), 0,
                             np.where((amin == 2) & (amax == 3), 2, 0)))
    ewid = np.where(amin == amax, 1,
                    np.where((amin == 0) & (amax == 1), 2,
                             np.where((amin == 2) & (amax == 3), 2, 4)))
    soff = np.concatenate([[0], np.cumsum(ewid)])   # block col offsets (32u)
    # decode (b, t, j)
    uj = ukeys % JMAX
    ut = ukeys // JMAX
    ub = bmap[ut]
    sched = {"b": ub, "t": ut, "j": uj, "col": ecol, "wid": ewid,
             "soff": soff, "n": TOTB, "totw": int(soff[-1])}

    # per-core S fill data (entry id + in-chunk row + in-block col per edge)
    for c in range(NCORES):
        d = cores[c]
        ent = np.searchsorted(ukeys, d["key"])
        d["ent"] = ent
        d["k"] = d["q"] % 128
        d["scol"] = d["rank"] % 128 - ecol[ent] * 32

    meta = {"G": G, "qoff": qoff, "goff": goff, "TOTQ": TOTQ,
            "TOTG": TOTG, "sched": sched, "pos": pos, "NBAT": NBAT,
            "btiles": btiles, "perm_nodes": perm_nodes, "cores": cores}
    return meta


def _build_s(meta):
    """Per-core fp8 S blocks, stream replication maps."""
    TOTQ = meta["TOTQ"]
    s_all, streams, spq_all = [], [], []
    for c in range(NCORES):
        d = meta["cores"][c]
        soff = meta["sched"]["soff"]
        totw = meta["sched"]["totw"]
        S = np.zeros((128, totw * 32), ml_dtypes.float8_e4m3)
        S[d["k"], soff[d["ent"]] * 32 + d["scol"]] = 1.0
        s_all.append(S)

        # per-position (p, g, rank) for the a streams
        streams.append((d["q"] % 128, d["q"] // 128, d["rank"]))

        # per-position source table row (0 for padding)
        spq = np.zeros(TOTQ, np.int64)
        spq[d["q"]] = d["sp"]
        spq_all.append(spq)
    return s_all, streams, spq_all


def _expand_stream(stream, r_core, width, totg):
    """r_core [MPC, width] f32 -> per-position [128, totg, width] f16."""
    p, g, rank = stream
    out = np.zeros((128, int(totg), width), np.float16)
    out[p, g, :] = r_core[rank, :width].astype(np.float16)
    return out


def _expand_rows(spq, table, width):
    """table [NROWS, width] f16 -> per-position stream [128, TOTG*width]."""
    rows = table[spq]                                # [TOTQ, width]
    return np.ascontiguousarray(
        rows.reshape(-1, 128, width).transpose(1, 0, 2)).reshape(128, -1)


# --------------------------------------------------------------------------
# launch builders
# --------------------------------------------------------------------------

def _new_nc():
    return bacc.Bacc("TRN2", target_bir_lowering=False, debug=False,
                     enable_asserts=False, num_devices=NCORES)


def _build_launch1():
    nc = _new_nc()
    xs_d = nc.dram_tensor("xs", [IN, MPC], F16, kind="ExternalInput")
    wc_d = nc.dram_tensor("wc", [IN, 80], F16, kind="ExternalInput")
    t1_d = nc.dram_tensor("t1s", [128, NT * NCOLS1], F16,
                          kind="ExternalOutput")
    r1_d = nc.dram_tensor("r1", [128, NT * 8], F32, kind="ExternalOutput")
    SLAB = SLAB1
    with tile.TileContext(nc) as tc:
        with tc.tile_pool(name="w", bufs=1) as wp, \
             tc.tile_pool(name="x", bufs=3) as xp, \
             tc.tile_pool(name="o", bufs=3) as op, \
             tc.tile_pool(name="ps", bufs=4, space="PSUM") as pp:
            wc_sb = wp.tile([128, 2, 80], F16)
            nc.sync.dma_start(wc_sb[:, 0, :], wc_d.ap()[0:128, :])
            nc.sync.dma_start(wc_sb[:, 1, :], wc_d.ap()[128:256, :])
            for s in range(NT // SLAB):
                cols = slice(s * SLAB * 128, (s + 1) * SLAB * 128)
                xt0 = xp.tile([128, SLAB * 128], F16, tag="xt0")
                xt1 = xp.tile([128, SLAB * 128], F16, tag="xt1")
                nc.sync.dma_start(xt0[:], xs_d.ap()[0:128, cols])
                nc.sync.dma_start(xt1[:], xs_d.ap()[128:256, cols])
                tout = op.tile([128, SLAB, NCOLS1], F16, tag="tout")
                rout = op.tile([128, SLAB, 8], F32, tag="rout")
                for half, (c0, c1) in enumerate(((0, 6), (6, 12), (12, SLAB))):
                    psb = pp.tile([128, c1 - c0, 80], F32, tag=f"ps{half}",
                                  bufs=2)
                    for i in range(c0, c1):
                        nc.tensor.matmul(
                            psb[:, i - c0, :],
                            lhsT=xt0[:, i * 128:(i + 1) * 128],
                            rhs=wc_sb[:, 0, :], start=True, stop=False,
                            skip_group_check=True)
                        nc.tensor.matmul(
                            psb[:, i - c0, :],
                            lhsT=xt1[:, i * 128:(i + 1) * 128],
                            rhs=wc_sb[:, 1, :], start=False, stop=True,
                            skip_group_check=True)
                    nc.vector.tensor_copy(tout[:, c0:c1, :],
                                          psb[:, :, 0:NCOLS1])
                    nc.scalar.copy(rout[:, c0:c1, :], psb[:, :, 72:80])
                c0 = s * SLAB
                nc.sync.dma_start(
                    t1_d.ap()[:, c0 * NCOLS1:(c0 + SLAB) * NCOLS1],
                    tout[:].rearrange("p i f -> p (i f)"))
                nc.scalar.dma_start(
                    r1_d.ap()[:, c0 * 8:(c0 + SLAB) * 8],
                    rout[:].rearrange("p i f -> p (i f)"))
    nc.compile()
    return nc


def _emit_msg_layer(nc, tc, meta, ms_d, s_d, ae_d, finalize,
                    rwidth, tabw, mwidth):
    """Shared structure of launches 2/3.

    rwidth: per-edge a-stream width (8 for L1, 1 for L2); tabw: stream row
    width; mwidth: matmul rhs width (= hcols + rwidth).  `finalize`
    supplies the per-edge elementwise ops and the per-batch epilogue.
    """
    G, qoff, goff = meta["G"], meta["qoff"], meta["goff"]
    sched = meta["sched"]
    sb, st, sj = (sched[k] for k in ("b", "t", "j"))
    scol, swid, soff = sched["col"], sched["wid"], sched["soff"]
    TOTB = sched["n"]
    ent_by_t = {}
    for i in range(TOTB):
        ent_by_t.setdefault(int(st[i]), []).append(i)
    NBAT = meta["NBAT"]
    btiles = meta["btiles"]
    blo = np.searchsorted(sb, np.arange(NBAT))
    bhi = np.searchsorted(sb, np.arange(NBAT), side="right")
    # batch S-column ranges (32-unit blocks)
    slo = [int(soff[blo[b]]) for b in range(NBAT)]
    shi = [int(soff[bhi[b]]) for b in range(NBAT)]
    nw32max = max(1, max(shi[b] - slo[b] for b in range(NBAT)))
    gmax = int(G.max())

    with tc.tile_pool(name="resident", bufs=1) as rp, \
         tc.tile_pool(name="gslab", bufs=3) as gp, \
         tc.tile_pool(name="mslab", bufs=1) as mp, \
         tc.tile_pool(name="work", bufs=3) as wkp, \
         tc.tile_pool(name="fin", bufs=3) as fp, \
         tc.tile_pool(name="psA", bufs=3, space="PSUM") as ppA, \
         tc.tile_pool(name="psB", bufs=2, space="PSUM") as ppB:
        pools = (rp, gp, mp, wkp, fp, ppA, ppB)
        zrow = rp.tile([1, 128], F16)
        nc.vector.memset(zrow[:], 0.0)
        zwide = rp.tile([1, 512], F16)
        nc.vector.memset(zwide[:], 0.0)
        cst_sb = finalize.load_consts(nc, rp)
        # finalize may aggregate SUPER consecutive batches into one PSUM
        # group (one epilogue pass); all tiles of a group are consecutive
        SUPER = getattr(finalize, "SUPER", 1)
        grp_of = [b // SUPER for b in range(NBAT)]
        NGRP = grp_of[-1] + 1
        gtiles = [sum((btiles[b] for b in range(NBAT) if grp_of[b] == gg),
                      []) for gg in range(NGRP)]
        snbmax = max(len(ts) for ts in gtiles)
        pending = None
        ps = None
        for b in range(NBAT):
            nb = len(btiles[b])
            gg = grp_of[b]
            g = int(G[b])
            g0 = int(goff[b])
            nw32 = max(shi[b] - slo[b], 1)
            ssb = mp.tile([128, nw32max, 32], F8, tag="s", bufs=3)
            if shi[b] > slo[b]:
                nc.sync.dma_start(
                    ssb[:, 0:nw32, :],
                    s_d.ap()[:, slo[b] * 32:shi[b] * 32]
                    .rearrange("p (n c) -> p n c", c=32))
            if b == 0 or grp_of[b - 1] != gg:
                snb = len(gtiles[gg])
                gt0 = int(gtiles[gg][0])
                ps = ppA.tile([128, snbmax, mwidth], F32, tag="ps")
                nc.tensor.matmul(
                    ps[:, 0:snb, :].rearrange("p n m -> p (n m)"),
                    lhsT=zrow[:], rhs=zwide[:, 0:snb * mwidth],
                    start=True, stop=False, skip_group_check=True)
            # per-edge message stream (host halo exchange), full bandwidth
            Gs = gp.tile([128, gmax, mwidth], F16, tag="G", bufs=3)
            nc.sync.dma_start(
                Gs[:, 0:g, 0:tabw],
                ms_d.ap()[:, g0 * tabw:(g0 + g) * tabw]
                .rearrange("p (g r) -> p g r", r=tabw))
            rs = gp.tile([128, gmax, rwidth], F16, tag="rs")
            nc.scalar.dma_start(
                rs[:, 0:g, :], ae_d.ap()[:, g0 * rwidth:(g0 + g) * rwidth]
                .rearrange("p (g r) -> p g r", r=rwidth))
            if DBG != "stream":
                finalize.edge_ops(nc, Gs, rs, wkp, g)
            if DBG in ("stream", "edge"):
                continue
            # previous group's epilogue goes between this batch's edge ops
            # and its matmuls: the DVE queue never stalls on the PE
            if DBG == "full" and pending is not None:
                finalize.batch_ops(nc, pools, *pending, cst_sb)
                pending = None
            # matmuls, tile-major within the batch
            for t in btiles[b]:
                i = int(t) - gt0
                ents = ent_by_t.get(t, [])
                for n, e in enumerate(ents):
                    j = int(sj[e])
                    col, wid = int(scol[e]), int(swid[e])
                    so = int(soff[e]) - slo[b]
                    nc.tensor.matmul(
                        ps[col * 32:(col + wid) * 32, i, :],
                        lhsT=ssb[:, so:so + wid, :]
                        .rearrange("p n c -> p (n c)"),
                        rhs=Gs[:, j, :],
                        start=False, stop=(n == len(ents) - 1),
                        tile_position=(0, col * 32),
                        skip_group_check=True)
            if b == NBAT - 1 or grp_of[b + 1] != gg:
                pending = (gg, gt0, snb, ps)
        if DBG == "full" and pending is not None:
            finalize.batch_ops(nc, pools, *pending, cst_sb)
        if DBG == "full" and hasattr(finalize, "finish"):
            finalize.finish(nc)


class _L1Final:
    """Layer-1 epilogue: softmax normalize, ELU, project through
    [W2 | W2 a_src2 | W2 a_dst2] to build the 17-col T2 row + ad2."""

    def __init__(self, nc, w2e_d, id_d, t2_d, r2_d):
        self.w2e_d, self.id_d = w2e_d, id_d
        self.t2_d, self.r2_d = t2_d, r2_d
        self.r2_sb = None

    def load_consts(self, nc, rp):
        # w2e replicated in both partition halves (matmul lhsT/rhs must
        # share a base partition)
        w2e = rp.tile([128, 18], F16)
        nc.sync.dma_start(w2e[0:64, :], self.w2e_d.ap())
        nc.sync.dma_start(w2e[64:128, :], self.w2e_d.ap())
        idm = rp.tile([128, 128], F32)
        nc.sync.dma_start(idm[:], self.id_d.ap())
        self.r2_sb = rp.tile([128, NT], F32)
        nc.vector.memset(self.r2_sb[:], 0.0)
        return (w2e, idm)

    def finish(self, nc):
        nc.gpsimd.dma_start(
            self.r2_d.ap().rearrange("(t p) o -> p (t o)", p=128), self.r2_sb[:])

    def edge_ops(self, nc, Gs, rs, wkp, g):
        # z = s[src] + a[dst]; el = exp(max(z, 0.2 z)); in-place h *= el.
        # z/zl run on the (otherwise idle) GPSIMD; the big multiply keeps
        # all last dims packed (h block is channel-major) for 2x DVE mode.
        z = wkp.tile([128, Gs.shape[1], 8], F16, tag="z")
        nc.vector.tensor_tensor(out=z[:, 0:g], in0=Gs[:, 0:g, 64:72],
                                in1=rs[:, 0:g], op=ALU.add)
        zl = wkp.tile([128, Gs.shape[1], 8], F16, tag="zl")
        nc.vector.tensor_scalar_mul(zl[:, 0:g], z[:, 0:g], NEG)
        nc.vector.tensor_tensor(out=zl[:, 0:g], in0=z[:, 0:g],
                                in1=zl[:, 0:g], op=ALU.max)
        nc.scalar.activation(out=Gs[:, 0:g, 64:72], in_=zl[:, 0:g],
                             func=AF.Exp)
        gh = (g + 1) // 2
        for lo, hi in ((0, gh), (gh, g)):
            gl = hi - lo
            nc.vector.tensor_tensor(
                out=Gs[:, lo:hi, 0:64].rearrange("p g (c h) -> p g c h", c=8),
                in0=Gs[:, lo:hi, 0:64].rearrange("p g (c h) -> p g c h", c=8),
                in1=Gs[:, lo:hi, 64:72].unsqueeze(2)
                .broadcast_to([128, gl, 8, 8]),
                op=ALU.mult)

    def batch_ops(self, nc, pools, gg, t0, nb, ps, consts):
        rp, gp, mp, wkp, fp, ppA, ppB = pools
        w2e, idm = consts
        den = fp.tile([128, NBMAX, 8], F32, tag="den")
        nc.vector.tensor_scalar_add(den[:, 0:nb], ps[:, 0:nb, 64:72], EPS)
        rec = fp.tile([128, NBMAX, 8], F32, tag="rec")
        nc.vector.reciprocal(rec[:, 0:nb], den[:, 0:nb])
        y = fp.tile([128, NBMAX, 64], F32, tag="y")
        nc.vector.tensor_tensor(
            out=y[:, 0:nb].rearrange("p n (c h) -> p n c h", c=8),
            in0=ps[:, 0:nb, 0:64].rearrange("p n (c h) -> p n c h", c=8),
            in1=rec[:, 0:nb].unsqueeze(2).broadcast_to([128, nb, 8, 8]),
            op=ALU.mult)
        # ELU: t2h = max(y, exp(min(y,0)) - 1)
        yn = fp.tile([128, NBMAX, 64], F32, tag="yn")
        nc.vector.tensor_scalar_min(yn[:, 0:nb], y[:, 0:nb], 0.0)
        ey = fp.tile([128, NBMAX, 64], F32, tag="ey")
        nc.scalar.activation(out=ey[:, 0:nb], in_=yn[:, 0:nb], func=AF.Exp)
        nc.vector.tensor_scalar_add(ey[:, 0:nb], ey[:, 0:nb], -1.0)
        t2h = fp.tile([128, NBMAX, 64], F32, tag="t2h")
        nc.vector.tensor_tensor(out=t2h[:, 0:nb], in0=y[:, 0:nb],
                                in1=ey[:, 0:nb], op=ALU.max)
        if FINLVL < 2:
            return
        # project tiles: transpose then @ [W2 | w2a | w2d]
        ps2 = ppB.tile([128, NBMAX, 18], F32, tag="ps2", bufs=2)
        for i in range(nb):
            tp = ppB.tile([64, 128], F32, tag="tp", bufs=2)
            nc.tensor.transpose(tp[:], t2h[:, i, :], idm[:])
            agT = fp.tile([64, 128], F16, tag="agT")
            nc.scalar.copy(agT[:], tp[:])
            if FINLVL < 3:
                continue
            nc.tensor.matmul(ps2[:, i, :], lhsT=agT[:], rhs=w2e[0:64, :],
                             start=True, stop=True, skip_group_check=True)
        if FINLVL < 4:
            return
        t2t = fp.tile([128, NBMAX, NCOLS2], F16, tag="t2t")
        nc.vector.tensor_copy(t2t[:, 0:nb, :], ps2[:, 0:nb, 0:18])
        nc.scalar.copy(self.r2_sb[:, t0:t0 + nb], ps2[:, 0:nb, 17])
        rows = slice(t0 * 128, (t0 + nb) * 128)
        nc.gpsimd.dma_start(
            self.t2_d.ap()[rows, :].rearrange("(i p) f -> p i f", p=128),
            t2t[:, 0:nb, :])


class _L2Final:
    """Layer-2 epilogue: normalize (already in W2-projected space),
    log_softmax.  SUPER consecutive batches share one PSUM group (28 x 18
    f32 rows fill exactly one PSUM bank), so the epilogue chain runs 4x
    less often and the output writes in >=512B fragments."""

    SUPER = 4
    SNB = NBMAX * SUPER

    def __init__(self, nc, o_d):
        self.o_d = o_d

    def load_consts(self, nc, rp):
        return None

    def edge_ops(self, nc, Gs, rs, wkp, g):
        # el duplicated into cols 16 and 17 so the 16-wide multiply keeps
        # packed last dims (2x DVE mode)
        z = wkp.tile([128, Gs.shape[1], 1], F16, tag="z")
        nc.vector.tensor_tensor(out=z[:, 0:g], in0=Gs[:, 0:g, 16:17],
                                in1=rs[:, 0:g], op=ALU.add)
        zl = wkp.tile([128, Gs.shape[1], 1], F16, tag="zl")
        nc.vector.tensor_scalar_mul(zl[:, 0:g], z[:, 0:g], NEG)
        nc.vector.tensor_tensor(out=zl[:, 0:g], in0=z[:, 0:g],
                                in1=zl[:, 0:g], op=ALU.max)
        nc.scalar.activation(out=Gs[:, 0:g, 16:17], in_=zl[:, 0:g],
                             func=AF.Exp)
        nc.scalar.activation(out=Gs[:, 0:g, 17:18], in_=zl[:, 0:g],
                             func=AF.Exp)
        nc.vector.tensor_tensor(
            out=Gs[:, 0:g, 0:16].rearrange("p g (u v) -> p g u v", v=2),
            in0=Gs[:, 0:g, 0:16].rearrange("p g (u v) -> p g u v", v=2),
            in1=Gs[:, 0:g, 16:18].unsqueeze(2).broadcast_to([128, g, 8, 2]),
            op=ALU.mult)

    def batch_ops(self, nc, pools, gg, t0, nb, ps, consts):
        rp, gp, mp, wkp, fp, ppA, ppB = pools
        SNB = self.SNB
        # den > 0 (self-loops); logits are small, so log_softmax needs no
        # max subtraction
        rec = fp.tile([128, SNB, 1], F32, tag="rec2")
        nc.vector.reciprocal(rec[:, 0:nb], ps[:, 0:nb, 16:17])
        o1 = fp.tile([128, SNB, 16], F32, tag="o1")
        nc.vector.tensor_tensor(
            out=o1[:, 0:nb], in0=ps[:, 0:nb, 0:16],
            in1=rec[:, 0:nb].rearrange("p n o -> p (n o)")
            .to_broadcast([128, nb, 16]),
            op=ALU.mult)
        es = fp.tile([128, SNB, 16], F16, tag="es")
        nc.scalar.activation(out=es[:, 0:nb], in_=o1[:, 0:nb], func=AF.Exp)
        ssum = fp.tile([128, SNB, 1], F32, tag="ssum")
        nc.vector.tensor_reduce(out=ssum[:, 0:nb], in_=es[:, 0:nb], axis=AX.X,
                                op=ALU.add)
        lns = fp.tile([128, SNB, 1], F32, tag="lns")
        nc.scalar.activation(out=lns[:, 0:nb], in_=ssum[:, 0:nb], func=AF.Ln)
        res = fp.tile([128, SNB, 16], F32, tag="res")
        nc.vector.tensor_tensor(
            out=res[:, 0:nb], in0=o1[:, 0:nb],
            in1=lns[:, 0:nb].rearrange("p n o -> p (n o)")
            .to_broadcast([128, nb, 16]),
            op=ALU.subtract)
        nc.gpsimd.dma_start(
            self.o_d.ap()[:, t0 * 16:(t0 + nb) * 16],
            res[:, 0:nb, :].rearrange("p n o -> p (n o)"))


def _build_launch2(meta):
    nc = _new_nc()
    ms_d = nc.dram_tensor("ms1", [128, meta["TOTG"] * NCOLS1], F16,
                          kind="ExternalInput")
    s_d = nc.dram_tensor("sall", [128, meta["sched"]["totw"] * 32], F8,
                         kind="ExternalInput")
    ae_d = nc.dram_tensor("ae1", [128, meta["TOTG"] * 8], F16,
                          kind="ExternalInput")
    w2e_d = nc.dram_tensor("w2e", [64, 18], F16, kind="ExternalInput")
    id_d = nc.dram_tensor("idm", [128, 128], F32, kind="ExternalInput")
    t2_d = nc.dram_tensor("t2s", [MPC, NCOLS2], F16, kind="ExternalOutput")
    r2_d = nc.dram_tensor("r2", [MPC, 1], F32, kind="ExternalOutput")
    fin = _L1Final(nc, w2e_d, id_d, t2_d, r2_d)
    with tile.TileContext(nc) as tc:
        _emit_msg_layer(nc, tc, meta, ms_d, s_d, ae_d, fin,
                        rwidth=8, tabw=NCOLS1, mwidth=MW1)
    nc.compile()
    return nc


def _build_launch3(meta):
    nc = _new_nc()
    ms_d = nc.dram_tensor("ms2", [128, meta["TOTG"] * NCOLS2], F16,
                          kind="ExternalInput")
    s_d = nc.dram_tensor("sall", [128, meta["sched"]["totw"] * 32], F8,
                         kind="ExternalInput")
    ae_d = nc.dram_tensor("ae2", [128, meta["TOTG"] * 1], F16,
                          kind="ExternalInput")
    o_d = nc.dram_tensor("o", [128, NT * 16], F32, kind="ExternalOutput")
    fin = _L2Final(nc, o_d)
    with tile.TileContext(nc) as tc:
        _emit_msg_layer(nc, tc, meta, ms_d, s_d, ae_d, fin,
                        rwidth=1, tabw=NCOLS2, mwidth=MW2)
    nc.compile()
    return nc


# --------------------------------------------------------------------------
# the kernel
# --------------------------------------------------------------------------

def kernel(x, edge_index, W1, a_src1, a_dst1, b1, W2, a_src2, a_dst2, b2):
    x = np.asarray(x, np.float32)
    edge_index = np.asarray(edge_index)
    W1 = np.asarray(W1, np.float32)
    W2 = np.asarray(W2, np.float32)
    a_src1 = np.asarray(a_src1, np.float32)
    a_dst1 = np.asarray(a_dst1, np.float32)
    a_src2 = np.asarray(a_src2, np.float32)
    a_dst2 = np.asarray(a_dst2, np.float32)

    key = edge_index.tobytes()[:4096]
    if _CACHE.get("key") != key:
        meta = _preprocess(edge_index)
        s_all, streams, spq_all = _build_s(meta)
        _CACHE.update(key=key, meta=meta, s_all=s_all, streams=streams,
                      spq_all=spq_all,
                      nc1=_build_launch1(), nc2=_build_launch2(meta),
                      nc3=_build_launch3(meta))
    meta = _CACHE["meta"]
    s_all, streams, spq_all = (_CACHE["s_all"], _CACHE["streams"],
                               _CACHE["spq_all"])

    # weight packing.  The h blocks live channel-major ((c, h) index) on
    # device so the per-edge el broadcast keeps packed last dims; reorder
    # W1 columns / W2 rows correspondingly here.
    W1r = W1.reshape(IN, HEADS, HID)
    B1 = np.einsum("khc,hc->kh", W1r, a_src1)        # [256, 8]
    C1 = np.einsum("khc,hc->kh", W1r, a_dst1)
    W1cm = W1r.transpose(0, 2, 1).reshape(IN, 64)
    wc = np.concatenate([W1cm, B1, C1], 1).astype(np.float16)  # [256, 80]
    W2cm = W2.reshape(HEADS, HID, OUT).transpose(1, 0, 2).reshape(64, OUT)
    w2a = W2cm @ a_src2[0]                            # [64]
    w2d = W2cm @ a_dst2[0]
    w2e = np.concatenate([W2cm, w2a[:, None], w2d[:, None]],
                         1).astype(np.float16)        # [64, 18]
    idm = np.eye(128, dtype=np.float32)

    # launch 1: build T1 slices
    perm = meta["perm_nodes"]
    xT = np.zeros((IN, NROWS), np.float16)
    real = perm >= 0
    xT[:, real] = x[perm[real]].astype(np.float16).T
    in1 = [{"xs": np.ascontiguousarray(xT[:, c * MPC:(c + 1) * MPC]),
            "wc": wc} for c in range(NCORES)]
    r1_res = bass_utils.run_bass_kernel_spmd(
        _CACHE["nc1"], in1, core_ids=list(range(NCORES)), trace=TRACE)
    T1 = np.concatenate(
        [r1_res.results[c]["t1s"].reshape(128, NT, NCOLS1)
         .transpose(1, 0, 2).reshape(MPC, NCOLS1) for c in range(NCORES)], 0)

    # launch 2: layer-1 message passing (host halo exchange -> streams)
    in2 = []
    for c in range(NCORES):
        ms1 = _expand_rows(spq_all[c], T1, NCOLS1)
        r1c = (r1_res.results[c]["r1"].reshape(128, NT, 8)
               .transpose(1, 0, 2).reshape(MPC, 8))
        ae1 = _expand_stream(streams[c], r1c, 8, meta["TOTG"])
        in2.append({"ms1": ms1, "sall": s_all[c],
                    "ae1": ae1.reshape(128, -1), "w2e": w2e, "idm": idm})
    r2_res = bass_utils.run_bass_kernel_spmd(
        _CACHE["nc2"], in2, core_ids=list(range(NCORES)), trace=TRACE)
    T2 = np.concatenate([r2_res.results[c]["t2s"] for c in range(NCORES)], 0)

    # launch 3: layer-2 + head
    in3 = []
    for c in range(NCORES):
        ms2 = _expand_rows(spq_all[c], T2, NCOLS2)
        ae2 = _expand_stream(streams[c], r2_res.results[c]["r2"], 1,
                             meta["TOTG"])
        in3.append({"ms2": ms2, "sall": s_all[c],
                    "ae2": ae2.reshape(128, -1)})
    r3_res = bass_utils.run_bass_kernel_spmd(
        _CACHE["nc3"], in3, core_ids=list(range(NCORES)), trace=TRACE)
    o_all = np.concatenate(
        [r3_res.results[c]["o"].reshape(128, NT, 16)
         .transpose(1, 0, 2).reshape(MPC, 16) for c in range(NCORES)], 0)

    out = o_all[meta["pos"][np.arange(N)]].astype(np.float32)
    _CACHE["exec_ns"] = [r.exec_time_ns for r in (r1_res, r2_res, r3_res)]
    _CACHE["profiles"] = [r.profile_json for r in (r1_res, r2_res, r3_res)]
    _CACHE["traces"] = [r.instructions_and_trace
                        for r in (r1_res, r2_res, r3_res)]
    return out


def predict_ns():
    """Cost-model (TimelineSim) per-launch predictions for cached programs."""
    from concourse.timeline_sim import TimelineSim
    out = []
    for k in ("nc1", "nc2", "nc3"):
        out.append(TimelineSim(_CACHE[k]).simulate())
    return out
